# revision 61
# baseline (speedup 1.0000x reference)
"""DispLoss Trainium2 kernel v2: 8-core SPMD, collective-free main path.

reference semantics:
  protos = sequential-EMA-update(prototypes, features, labels)   # normalize each updated row
  logits = protos @ protos.T / 0.1
  loss   = mean(log((exp(logits).sum(1) - diag(exp(logits))) / (C-1)))

Strategy (per core, all redundant except the row-sharded matmul):
  - prototypes are replicated inputs, so every core builds the full transposed
    matrix AT locally: stream all 64 row-tiles of the OLD prototypes from HBM,
    multiply by a 0/1 per-row mask that zeroes rows the EMA will update
    (fused with the f32->bf16 convert), and PE-transpose into AT.
  - every core redundantly computes the EMA update for ALL ~u distinct labels
    (grouped into sequential rounds by occurrence index, routed through a
    small DRAM scratch E_dram), then appends the updated rows as 1024 extra
    "slot" columns of AT (unified domain: 9216 = 8192 + 1024 columns).
  - zeroed stale columns and zero pad columns each contribute exp(0) = 1 to a
    row's sum: exactly 1024 per row, subtracted as a constant.
  - rows are sharded: core c owns unified rows [c*1152, (c+1)*1152), gathered
    as merged tiles (old rows from protos, updated rows from E_dram, selected
    by per-core masks) and transposed into a per-core LHS buffer. Rows that
    are stale (their final value lives in a slot column) are zeroed and
    excluded from the mean via a row mask; exactly 8192 valid rows remain.
  - diag(exp) per row is exp(10 * |row|^2) computed from the same bf16 merged
    tiles (square + reduce), matching the matmul's self-dot exactly.
  - the only collective is the final scalar AllReduce.
"""
import sys
import types
import math
import numpy as np

sys.path.insert(0, "/opt/trn_rl_repo")

N_CORES = 8
C = 8192          # n_class
D = 1024          # feat_dim
B = 1024          # batch
KCH = D // 128    # feature chunks
SLOT = 1024       # slot-column region (max distinct labels)
CEXT = C + SLOT   # 9216 unified columns
OWN = CEXT // N_CORES   # 1152 rows per core
NT_OWN = OWN // 128     # 9 own row tiles
PANELS = CEXT // 1024   # 9 column panels
EDRAM_ROWS = 1280
TRASH = SLOT + 128      # 1152 (keeps indirect-scatter dest views small)
TEMP = 0.1
INV_TEMP = 1.0 / TEMP


def _shim_axon_hooks():
    try:
        from antenv.axon_hooks import get_axon_ntff_profile_hook  # noqa: F401
    except Exception:
        shim = types.ModuleType("antenv.axon_hooks")
        shim.get_axon_ntff_profile_hook = lambda: None
        sys.modules["antenv.axon_hooks"] = shim


def build_nc(round_tiles, loop_k=None, parts="full"):
    """Build the SPMD Bass program.

    round_tiles: global tile counts per EMA round, e.g. [8, 1, 1].
    loop_k: if set, wrap the compute body in For_i (timing builds; the final
            AllGather stays outside the loop).
    parts: "full" | "mm" (matmul phase only) | "prefix" (everything but the
           matmul phase) — timing-bisection builds only.
    """
    import concourse.mybir as mybir
    from concourse import bacc
    from concourse.tile import TileContext
    from concourse.bass import IndirectOffsetOnAxis
    from concourse.masks import make_identity
    from contextlib import ExitStack

    F32, BF16, I32 = mybir.dt.float32, mybir.dt.bfloat16, mybir.dt.int32
    AX = mybir.AxisListType
    OP = mybir.AluOpType
    AF = mybir.ActivationFunctionType

    nc = bacc.Bacc(None, target_bir_lowering=False, num_devices=N_CORES)

    feats = nc.declare_dram_parameter("features", [B, D], F32, isOutput=False)
    protos = nc.declare_dram_parameter("prototypes", [C, D], F32, isOutput=False)
    # global (same data on every core)
    zmask = nc.declare_dram_parameter("zmask", [128, C // 128], F32, isOutput=False)
    ridx = []
    for r, nt in enumerate(round_tiles):
        g = nc.declare_dram_parameter(f"g{r}", [128, nt], I32, isOutput=False)
        f = nc.declare_dram_parameter(f"f{r}", [128, nt], I32, isOutput=False)
        s = nc.declare_dram_parameter(f"s{r}", [128, nt], I32, isOutput=False)
        ridx.append((g, f, s))
    m0 = nc.declare_dram_parameter("m0", [128, round_tiles[0]], F32,
                                   isOutput=False)
    # per-core
    gp = nc.declare_dram_parameter("gp", [128, NT_OWN], I32, isOutput=False)
    gs = nc.declare_dram_parameter("gs", [128, NT_OWN], I32, isOutput=False)
    m1 = nc.declare_dram_parameter("m1", [128, NT_OWN], F32, isOutput=False)
    m2 = nc.declare_dram_parameter("m2", [128, NT_OWN], F32, isOutput=False)
    out = nc.declare_dram_parameter("out", [1, 1], F32, isOutput=True)

    with TileContext(nc) as tc:
        with ExitStack() as top:
            dram = top.enter_context(tc.tile_pool(name="dram", bufs=1, space="DRAM"))
            E_dram = dram.tile([EDRAM_ROWS, D], BF16)
            # compact per-round staging (round r>0 gathers its sources from
            # round r-1's staging, off the E_dram scatter critical path)
            E_stage = [None]
            for r in range(1, len(round_tiles)):
                E_stage.append(dram.tile([round_tiles[r] * 128, D], BF16,
                                         name=f"E_stage{r}"))
            ar_in = dram.tile([1, 1], F32)
            ar_out = dram.tile([N_CORES, 1], F32, addr_space="Shared")

            persist = top.enter_context(tc.tile_pool(name="persist", bufs=1))
            # AT: full transposed matrix, [feat%128, feat//128, unified col] bf16
            AT = persist.tile([128, KCH, CEXT], BF16)
            # LHS: own rows transposed, [feat%128, feat//128, own row] bf16
            LHS = persist.tile([128, KCH, OWN], BF16)
            ident = persist.tile([128, 128], BF16)
            make_identity(nc, ident[:])
            ones = persist.tile([128, 1], F32)
            nc.vector.memset(ones[:], 1.0)
            rowparts = persist.tile([128, NT_OWN, PANELS], F32)
            dlog = persist.tile([128, NT_OWN], F32)
            dexp = persist.tile([128, NT_OWN], F32)

            # index/mask inputs -> SBUF
            idxp = top.enter_context(tc.tile_pool(name="idx", bufs=1))
            zmask_sb = idxp.tile([128, C // 128], F32)
            nc.sync.dma_start(out=zmask_sb[:], in_=zmask[:, :])
            gp_sb = idxp.tile([128, NT_OWN], I32)
            gs_sb = idxp.tile([128, NT_OWN], I32)
            m1_sb = idxp.tile([128, NT_OWN], F32)
            m2_sb = idxp.tile([128, NT_OWN], F32)
            m0_sb = idxp.tile([128, round_tiles[0]], F32)
            nc.scalar.dma_start(out=gp_sb[:], in_=gp[:, :])
            nc.scalar.dma_start(out=gs_sb[:], in_=gs[:, :])
            nc.scalar.dma_start(out=m1_sb[:], in_=m1[:, :])
            nc.scalar.dma_start(out=m2_sb[:], in_=m2[:, :])
            nc.scalar.dma_start(out=m0_sb[:], in_=m0[:, :])
            ridx_sb = []
            for r, (g, f, s) in enumerate(ridx):
                nt = round_tiles[r]
                gsb = idxp.tile([128, nt], I32, name=f"g{r}sb")
                fsb = idxp.tile([128, nt], I32, name=f"f{r}sb")
                ssb = idxp.tile([128, nt], I32, name=f"s{r}sb")
                eng = nc.sync if r == 0 else nc.scalar
                eng.dma_start(out=gsb[:], in_=g[:, :])
                eng.dma_start(out=fsb[:], in_=f[:, :])
                eng.dma_start(out=ssb[:], in_=s[:, :])
                ridx_sb.append((gsb, fsb, ssb))

            def body():
                with ExitStack() as ph:
                    psv = ph.enter_context(
                        tc.tile_pool(name="psV", bufs=2, space="PSUM"))
                    sbe = ph.enter_context(tc.tile_pool(name="ema", bufs=2))
                    sbv = ph.enter_context(tc.tile_pool(name="phaseV", bufs=2))
                    pmm = ph.enter_context(
                        tc.tile_pool(name="psMM", bufs=3, space="PSUM"))

                    # ------------- Phase Z: zero E_dram trash region -----------
                    # (rows 0..SLOT are fully written by r0's direct stores,
                    # with pad rows zeroed via the m0 mask)
                    zeros_t = sbv.tile([128, D], BF16, tag="ck", name="zeros_t")
                    nc.vector.memset(zeros_t[:], 0.0)
                    for t in range(round_tiles[0], EDRAM_ROWS // 128):
                        nc.scalar.dma_start(
                            out=E_dram[t * 128:(t + 1) * 128, :], in_=zeros_t[:])

                    def ema_gather(r, t):
                        # indirect gathers for one EMA tile
                        gsb, fsb, ssb = ridx_sb[r]
                        if r == 0:
                            G = sbe.tile([128, D], F32, tag="gema",
                                         name=f"g_{r}_{t}")
                            nc.gpsimd.indirect_dma_start(
                                out=G[:, :], out_offset=None,
                                in_=protos[:, :],
                                in_offset=IndirectOffsetOnAxis(
                                    ap=gsb[:, t:t + 1], axis=0),
                            )
                        else:
                            G = sbe.tile([128, D], BF16, tag="mn",
                                         name=f"g_{r}_{t}")
                            gsrc = E_dram[0:SLOT, :] if r == 1 \
                                else E_stage[r - 1][:, :]
                            nc.gpsimd.indirect_dma_start(
                                out=G[:, :], out_offset=None,
                                in_=gsrc,
                                in_offset=IndirectOffsetOnAxis(
                                    ap=gsb[:, t:t + 1], axis=0),
                            )
                        Ft = sbe.tile([128, D], F32, tag="fema",
                                      name=f"f_{r}_{t}")
                        nc.gpsimd.indirect_dma_start(
                            out=Ft[:, :], out_offset=None,
                            in_=feats[:, :],
                            in_offset=IndirectOffsetOnAxis(
                                ap=fsb[:, t:t + 1], axis=0),
                        )
                        return G, Ft

                    def ema_compute(r, t, G, Ft):
                        # normalize + scatter for one EMA tile.  Buffer
                        # economy: Ft is squashed in place into m; G doubles
                        # as the m^2 dump (r0).
                        gsb, fsb, ssb = ridx_sb[r]
                        # m = f*(0.05/0.95) + g (positive scale of ref row;
                        # removed by the normalize)
                        nc.vector.scalar_tensor_tensor(
                            out=Ft[:], in0=Ft[:], scalar=0.05 / 0.95,
                            in1=G[:], op0=OP.mult, op1=OP.add,
                        )
                        if r == 0:
                            Msq = G  # G is dead; reuse as the square dump
                        else:
                            Msq = sbe.tile([128, D], F32, tag="gema",
                                           name=f"msq_{r}_{t}")
                        ssq = sbe.tile([128, 1], F32, tag="ssq")
                        nc.vector.scalar_tensor_tensor(
                            out=Msq[:], in0=Ft[:], scalar=1.0, in1=Ft[:],
                            op0=OP.mult, op1=OP.mult, accum_out=ssq[:],
                        )
                        rsq = sbe.tile([128, 1], F32, tag="rsq")
                        nc.vector.reciprocal(rsq[:], ssq[:])
                        rnorm = sbe.tile([128, 1], F32, tag="rnorm")
                        nc.scalar.activation(rnorm[:], rsq[:], AF.Sqrt)
                        Mn = sbe.tile([128, D], BF16, tag="mn",
                                      name=f"mn_{r}_{t}")
                        if r == 0:
                            # r0 slots are contiguous (first-occurrence
                            # order): zero the pad rows via m0 and store with
                            # a plain direct DMA instead of a scatter
                            nc.vector.tensor_scalar(
                                out=Mn[:], in0=Ft[:], scalar1=rnorm[:],
                                scalar2=m0_sb[:, t:t + 1],
                                op0=OP.mult, op1=OP.mult,
                            )
                            nc.scalar.dma_start(
                                out=E_dram[t * 128:(t + 1) * 128, :],
                                in_=Mn[:])
                        else:
                            nc.vector.tensor_scalar_mul(Mn[:], Ft[:], rnorm[:])
                            # compact staging feeds the next round directly;
                            # the E_dram scatter is off that critical path
                            nc.scalar.dma_start(
                                out=E_stage[r][t * 128:(t + 1) * 128, :],
                                in_=Mn[:])
                            nc.gpsimd.indirect_dma_start(
                                out=E_dram[0:TRASH + 1, :],
                                out_offset=IndirectOffsetOnAxis(
                                    ap=ssb[:, t:t + 1], axis=0),
                                in_=Mn[:, :], in_offset=None,
                            )

                    def ema_round(r):
                        pend = []
                        for t in range(round_tiles[r]):
                            pend.append((t,) + ema_gather(r, t))
                            if len(pend) > 1:
                                tt, G, Ft = pend.pop(0)
                                ema_compute(r, tt, G, Ft)
                        for tt, G, Ft in pend:
                            ema_compute(r, tt, G, Ft)

                    def transpose_tile(src_bf16, dst, dst_base, eng=nc.scalar):
                        # 8 PE transposes -> 1 psum [128,8,128] -> 1 strided copy
                        pt = psv.tile([128, KCH, 128], BF16, tag="pt")
                        for k in range(KCH):
                            nc.tensor.transpose(
                                pt[:, k, :],
                                src_bf16[:, k * 128:(k + 1) * 128],
                                ident[:],
                            )
                        dstv = dst[:, :, dst_base:dst_base + 128]
                        if eng is nc.vector:
                            nc.vector.tensor_copy(dstv, pt[:, :, :])
                        else:
                            nc.scalar.copy(dstv, pt[:, :, :])

                    def stage_tile(p, i, late=False):
                        # load + convert + transpose for tile i of panel p.
                        # late=True (inside the matmul window): converts run
                        # on gpsimd, which is idle once the prefix's indirect
                        # DMAs are done, leaving Act with only the exps.
                        if True:
                            t = p * 8 + i
                            if t < C // 128:
                                Lk = sbv.tile([128, D], F32, tag="lk")
                                nc.sync.dma_start(
                                    out=Lk[:],
                                    in_=protos[t * 128:(t + 1) * 128, :])
                                Ck = sbv.tile([128, D], BF16, tag="ck")
                                if late:
                                    nc.gpsimd.tensor_scalar_mul(
                                        Ck[:], Lk[:], zmask_sb[:, t:t + 1])
                                else:
                                    nc.scalar.mul(Ck[:], Lk[:],
                                                  zmask_sb[:, t:t + 1])
                                transpose_tile(Ck, AT, t * 128,
                                               eng=nc.vector)
                            else:
                                ts = t - C // 128
                                St = sbv.tile([128, D], BF16, tag="ck",
                                              name=f"st{ts}")
                                nc.gpsimd.dma_start(
                                    out=St[:],
                                    in_=E_dram[ts * 128:(ts + 1) * 128, :])
                                transpose_tile(St, AT, t * 128,
                                               eng=nc.vector)

                    def stage(p):
                        for i in range(8):
                            stage_tile(p, i)

                    def phase_L():
                        # merged own rows -> LHS + diag (tiles alias phaseV /
                        # ema pool tags; phase L strictly follows EMA)
                        for t in range(NT_OWN):
                            A = sbv.tile([128, D], F32, tag="lk",
                                         name=f"argh{t}")
                            nc.gpsimd.indirect_dma_start(
                                out=A[:, :], out_offset=None,
                                in_=protos[:, :],
                                in_offset=IndirectOffsetOnAxis(
                                    ap=gp_sb[:, t:t + 1], axis=0),
                            )
                            Bt = sbv.tile([128, D], BF16, tag="ck",
                                          name=f"bgh{t}")
                            nc.gpsimd.indirect_dma_start(
                                out=Bt[:, :], out_offset=None,
                                in_=E_dram[:, :],
                                in_offset=IndirectOffsetOnAxis(
                                    ap=gs_sb[:, t:t + 1], axis=0),
                            )
                            # bf16(A*m1) matches the AT path's bf16(zmask*row)
                            T1 = sbe.tile([128, D], BF16, tag="mn",
                                          name=f"t1_{t}")
                            nc.vector.tensor_scalar_mul(
                                T1[:], A[:], m1_sb[:, t:t + 1])
                            Mg = sbe.tile([128, D], BF16, tag="mn",
                                          name=f"mg{t}")
                            nc.vector.scalar_tensor_tensor(
                                out=Mg[:], in0=Bt[:],
                                scalar=m2_sb[:, t:t + 1],
                                in1=T1[:], op0=OP.mult, op1=OP.add,
                            )
                            transpose_tile(Mg, LHS, t * 128, eng=nc.vector)
                            # diag: |row|^2 of the same bf16 values (A is dead;
                            # reuse as the square dump, only the accum matters)
                            nc.vector.scalar_tensor_tensor(
                                out=A[:], in0=Mg[:], scalar=1.0, in1=Mg[:],
                                op0=OP.mult, op1=OP.mult,
                                accum_out=dlog[:, t:t + 1],
                            )

                    def mm_panel(p, stage_p=None):
                        # matmuls + exp row-sums for column panel p; the next
                        # panel's stage tiles interleave between matmul pairs
                        # so their transposes never starve PE
                        for j, b0 in enumerate(range(0, NT_OWN, 2)):
                            pair = [b0] if b0 + 1 >= NT_OWN else [b0, b0 + 1]
                            pss = []
                            for bi in pair:
                                ps = pmm.tile([128, 1024], F32, tag="ps",
                                              name=f"ps_{p}_{bi}")
                                pss.append(ps)
                            for k in range(KCH):
                                # 512-wide matmuls (ISA cap), 2x2 interleaved
                                # accumulation chains to hide psum latency
                                for h in range(2):
                                    for ps, bi in zip(pss, pair):
                                        nc.tensor.matmul(
                                            ps[:, h * 512:(h + 1) * 512],
                                            LHS[:, k, bi * 128:(bi + 1) * 128],
                                            AT[:, k, p * 1024 + h * 512:
                                               p * 1024 + (h + 1) * 512],
                                            start=(k == 0),
                                            stop=(k == KCH - 1),
                                        )
                            for ps, bi in zip(pss, pair):
                                # exp in place on the psum tile; only the
                                # accumulated row-sum is consumed
                                nc.scalar.activation(
                                    ps[:], ps[:], AF.Exp, scale=INV_TEMP,
                                    accum_out=rowparts[:, bi, p:p + 1],
                                )
                            if stage_p is not None:
                                for i in (2 * j, 2 * j + 1):
                                    if i < 8:
                                        stage_tile(stage_p, i, late=True)

                    # Emission: EMA rounds first (gathers pipelined one tile
                    # ahead of computes so scatters never head-block the
                    # gpsimd queue); stage 0 rides the DMA queue behind r0;
                    # phase L gates the first matmul; stages 1+ interleave
                    # with the matmul panels so their DMA hides under PE
                    # streaming.
                    # r0 tiles interleave with stage-0 tiles (r0 has no
                    # gpsimd scatters, so nothing head-blocks); gathers run
                    # one tile ahead of computes
                    if parts in ("full", "prefix"):
                        pend = []
                        for i in range(8):
                            if i < round_tiles[0]:
                                pend.append((i,) + ema_gather(0, i))
                            if len(pend) > 1:
                                tt, G, Ft = pend.pop(0)
                                ema_compute(0, tt, G, Ft)
                            stage_tile(0, i)
                        for tt, G, Ft in pend:
                            ema_compute(0, tt, G, Ft)
                        for r in range(1, len(round_tiles)):
                            ema_round(r)
                        phase_L()
                    if parts == "full":
                        for p in range(PANELS):
                            mm_panel(p, stage_p=p + 1 if p + 1 < PANELS
                                     else None)
                    elif parts == "mm":
                        for p in range(PANELS):
                            mm_panel(p, stage_p=None)
                    else:
                        for p in range(1, PANELS):
                            stage(p)
                    nc.scalar.activation(dexp[:], dlog[:], AF.Exp,
                                         scale=INV_TEMP)

                # ---------------- Phase F: local reduce ------------------------
                with ExitStack() as ph:
                    fin = ph.enter_context(tc.tile_pool(name="fin", bufs=1))
                    pfin = ph.enter_context(
                        tc.tile_pool(name="psFin", bufs=1, space="PSUM"))
                    rs = fin.tile([128, NT_OWN], F32)
                    nc.vector.tensor_reduce(
                        rs[:], rowparts[:, :, :], axis=AX.X, op=OP.add)
                    rmask = fin.tile([128, NT_OWN], F32)
                    nc.vector.tensor_add(rmask[:], m1_sb[:], m2_sb[:])
                    sn = fin.tile([128, NT_OWN], F32)
                    nc.vector.tensor_sub(sn[:], rs[:], dexp[:])
                    sn2 = fin.tile([128, NT_OWN], F32)
                    nc.vector.tensor_scalar_add(sn2[:], sn[:], -float(SLOT))
                    lg = fin.tile([128, NT_OWN], F32)
                    nc.scalar.activation(lg[:], sn2[:], AF.Ln)
                    lgm = fin.tile([128, NT_OWN], F32)
                    nc.vector.tensor_mul(lgm[:], lg[:], rmask[:])
                    l1 = fin.tile([128, 1], F32)
                    nc.vector.reduce_sum(l1[:], lgm[:], axis=AX.X)
                    pt1 = pfin.tile([1, 1], F32)
                    nc.tensor.matmul(pt1[:], ones[:], l1[:], start=True, stop=True)
                    tot = fin.tile([1, 1], F32)
                    nc.vector.tensor_copy(tot[:], pt1[:])
                    nc.sync.dma_start(out=ar_in[:, :], in_=tot[:])

            if loop_k is not None:
                with tc.For_i(0, loop_k, 1):
                    body()
            else:
                body()

            # ------------- AllGather partials + local sum + final scale ----
            nc.gpsimd.collective_compute(
                "AllGather", mybir.AluOpType.bypass,
                replica_groups=[list(range(N_CORES))],
                ins=[ar_in.opt()], outs=[ar_out.opt()],
            )
            with ExitStack() as ph:
                fin2 = ph.enter_context(tc.tile_pool(name="fin2", bufs=1))
                ld = fin2.tile([1, N_CORES], F32)
                nc.sync.dma_start(
                    out=ld[:], in_=ar_out[:, :].rearrange("a b -> b a"))
                tot8 = fin2.tile([1, 1], F32)
                nc.vector.reduce_sum(tot8[:], ld[:], axis=mybir.AxisListType.X)
                fv = fin2.tile([1, 1], F32)
                # loss = total/C - log(C-1)
                nc.vector.tensor_scalar(
                    out=fv[:], in0=tot8[:], scalar1=1.0 / C,
                    scalar2=-math.log(C - 1), op0=mybir.AluOpType.mult,
                    op1=mybir.AluOpType.add,
                )
                nc.sync.dma_start(out=out[:, :], in_=fv[:])

    nc.finalize()
    return nc


def host_prep(labels):
    """Host-side index/mask prep (global EMA rounds + per-core row shards)."""
    labels = np.asarray(labels).astype(np.int64)
    Bn = labels.shape[0]

    slot_of = {}          # label -> slot (first-occurrence order)
    occ = {}              # label -> #occurrences seen so far
    rounds = []           # rounds[r] = list of (slot, label, sample_idx)
    for i, l in enumerate(labels.tolist()):
        r = occ.get(l, 0)
        occ[l] = r + 1
        if l not in slot_of:
            slot_of[l] = len(slot_of)
        while len(rounds) <= r:
            rounds.append([])
        rounds[r].append((slot_of[l], l, i))
    u = len(slot_of)
    n_rounds = len(rounds)

    round_tiles = [max(1, (len(rounds[r]) + 127) // 128) for r in range(n_rounds)]

    def col_major(vec, nt):
        return np.ascontiguousarray(
            np.asarray(vec, dtype=np.int32).reshape(nt, 128).T)

    glob = {}
    stale = np.zeros(C, dtype=bool)
    for l in slot_of:
        stale[l] = True
    zm = np.where(stale, 0.0, 1.0).astype(np.float32)
    glob["zmask"] = np.ascontiguousarray(zm.reshape(C // 128, 128).T)

    for r in range(n_rounds):
        L = round_tiles[r] * 128
        g = np.zeros(L, dtype=np.int32)
        f = np.zeros(L, dtype=np.int32)
        s = np.full(L, TRASH, dtype=np.int32)
        if r >= 2:
            # position of each label within the previous round's staging
            pos_prev = {l: j for j, (_, l, _) in enumerate(rounds[r - 1])}
        for j, (slot, l, i) in enumerate(rounds[r]):
            if r == 0:
                g[j] = l          # prototypes row
            elif r == 1:
                g[j] = slot       # E_dram row (round-0 result)
            else:
                g[j] = pos_prev[l]  # E_stage[r-1] row
            f[j] = i
            s[j] = slot
        glob[f"g{r}"] = col_major(g, round_tiles[r])
        glob[f"f{r}"] = col_major(f, round_tiles[r])
        glob[f"s{r}"] = col_major(s, round_tiles[r])

    L0 = round_tiles[0] * 128
    m0v = (np.arange(L0) < len(rounds[0])).astype(np.float32)
    glob["m0"] = np.ascontiguousarray(
        m0v.reshape(round_tiles[0], 128).T)

    per_core = []
    for c in range(N_CORES):
        gp = np.zeros(OWN, dtype=np.int32)
        gs = np.full(OWN, TRASH, dtype=np.int32)
        m1v = np.zeros(OWN, dtype=np.float32)
        m2v = np.zeros(OWN, dtype=np.float32)
        for j in range(OWN):
            row = c * OWN + j
            if row < C:
                gp[j] = row
                if not stale[row]:
                    m1v[j] = 1.0
            else:
                slot = row - C
                gs[j] = slot
                if slot < u:
                    m2v[j] = 1.0
        d = dict(glob)
        d["gp"] = col_major(gp, NT_OWN)
        d["gs"] = col_major(gs, NT_OWN)
        d["m1"] = np.ascontiguousarray(
            m1v.reshape(NT_OWN, 128).T.astype(np.float32))
        d["m2"] = np.ascontiguousarray(
            m2v.reshape(NT_OWN, 128).T.astype(np.float32))
        per_core.append(d)
    return round_tiles, per_core


_NC_CACHE = {}


def kernel(features, labels, prototypes):
    _shim_axon_hooks()
    from concourse.bass_utils import run_bass_kernel_spmd

    features = np.ascontiguousarray(np.asarray(features), dtype=np.float32)
    prototypes = np.ascontiguousarray(np.asarray(prototypes), dtype=np.float32)
    round_tiles, per_core = host_prep(labels)

    key = tuple(round_tiles)
    if key not in _NC_CACHE:
        _NC_CACHE[key] = build_nc(round_tiles)
    nc = _NC_CACHE[key]

    in_maps = []
    for c in range(N_CORES):
        m = {"features": features, "prototypes": prototypes}
        m.update(per_core[c])
        in_maps.append(m)

    res = run_bass_kernel_spmd(nc, in_maps, core_ids=list(range(N_CORES)), trace=False)
    val = np.float32(res.results[0]["out"][0, 0])
    return np.asarray(val, dtype=np.float32).reshape(())


# revision 64
# speedup vs baseline: 1.0110x; 1.0110x over previous
"""DispLoss Trainium2 kernel v2: 8-core SPMD, collective-free main path.

reference semantics:
  protos = sequential-EMA-update(prototypes, features, labels)   # normalize each updated row
  logits = protos @ protos.T / 0.1
  loss   = mean(log((exp(logits).sum(1) - diag(exp(logits))) / (C-1)))

Strategy (per core, all redundant except the row-sharded matmul):
  - prototypes are replicated inputs, so every core builds the full transposed
    matrix AT locally: stream all 64 row-tiles of the OLD prototypes from HBM,
    multiply by a 0/1 per-row mask that zeroes rows the EMA will update
    (fused with the f32->bf16 convert), and PE-transpose into AT.
  - every core redundantly computes the EMA update for ALL ~u distinct labels
    (grouped into sequential rounds by occurrence index, routed through a
    small DRAM scratch E_dram), then appends the updated rows as 1024 extra
    "slot" columns of AT (unified domain: 9216 = 8192 + 1024 columns).
  - zeroed stale columns and zero pad columns each contribute exp(0) = 1 to a
    row's sum: exactly 1024 per row, subtracted as a constant.
  - rows are sharded: core c owns unified rows [c*1152, (c+1)*1152), gathered
    as merged tiles (old rows from protos, updated rows from E_dram, selected
    by per-core masks) and transposed into a per-core LHS buffer. Rows that
    are stale (their final value lives in a slot column) are zeroed and
    excluded from the mean via a row mask; exactly 8192 valid rows remain.
  - diag(exp) per row is exp(10 * |row|^2) computed from the same bf16 merged
    tiles (square + reduce), matching the matmul's self-dot exactly.
  - the only collective is the final scalar AllReduce.
"""
import sys
import types
import math
import numpy as np

sys.path.insert(0, "/opt/trn_rl_repo")

N_CORES = 8
C = 8192          # n_class
D = 1024          # feat_dim
B = 1024          # batch
KCH = D // 128    # feature chunks
SLOT = 1024       # slot-column region (max distinct labels)
CEXT = C + SLOT   # 9216 unified columns
OWN = CEXT // N_CORES   # 1152 rows per core
NT_OWN = OWN // 128     # 9 own row tiles
PANELS = CEXT // 1024   # 9 column panels
EDRAM_ROWS = 1280
TRASH = SLOT + 128      # 1152 (keeps indirect-scatter dest views small)
TEMP = 0.1
INV_TEMP = 1.0 / TEMP


def _shim_axon_hooks():
    try:
        from antenv.axon_hooks import get_axon_ntff_profile_hook  # noqa: F401
    except Exception:
        shim = types.ModuleType("antenv.axon_hooks")
        shim.get_axon_ntff_profile_hook = lambda: None
        sys.modules["antenv.axon_hooks"] = shim


def build_nc(round_tiles, loop_k=None, parts="full"):
    """Build the SPMD Bass program.

    round_tiles: global tile counts per EMA round, e.g. [8, 1, 1].
    loop_k: if set, wrap the compute body in For_i (timing builds; the final
            AllGather stays outside the loop).
    parts: "full" | "mm" (matmul phase only) | "prefix" (everything but the
           matmul phase) — timing-bisection builds only.
    """
    import concourse.mybir as mybir
    from concourse import bacc
    from concourse.tile import TileContext
    from concourse.bass import IndirectOffsetOnAxis
    from concourse.masks import make_identity
    from contextlib import ExitStack

    F32, BF16, I32 = mybir.dt.float32, mybir.dt.bfloat16, mybir.dt.int32
    AX = mybir.AxisListType
    OP = mybir.AluOpType
    AF = mybir.ActivationFunctionType

    nc = bacc.Bacc(None, target_bir_lowering=False, num_devices=N_CORES)

    feats = nc.declare_dram_parameter("features", [B, D], F32, isOutput=False)
    protos = nc.declare_dram_parameter("prototypes", [C, D], F32, isOutput=False)
    # global (same data on every core)
    zmask = nc.declare_dram_parameter("zmask", [128, C // 128], F32, isOutput=False)
    ridx = []
    for r, nt in enumerate(round_tiles):
        g = nc.declare_dram_parameter(f"g{r}", [128, nt], I32, isOutput=False)
        f = nc.declare_dram_parameter(f"f{r}", [128, nt], I32, isOutput=False)
        s = nc.declare_dram_parameter(f"s{r}", [128, nt], I32, isOutput=False)
        ridx.append((g, f, s))
    m0 = nc.declare_dram_parameter("m0", [128, round_tiles[0]], F32,
                                   isOutput=False)
    # per-core
    gp = nc.declare_dram_parameter("gp", [128, NT_OWN], I32, isOutput=False)
    gs = nc.declare_dram_parameter("gs", [128, NT_OWN], I32, isOutput=False)
    m1 = nc.declare_dram_parameter("m1", [128, NT_OWN], F32, isOutput=False)
    m2 = nc.declare_dram_parameter("m2", [128, NT_OWN], F32, isOutput=False)
    out = nc.declare_dram_parameter("out", [1, 1], F32, isOutput=True)

    with TileContext(nc) as tc:
        with ExitStack() as top:
            dram = top.enter_context(tc.tile_pool(name="dram", bufs=1, space="DRAM"))
            E_dram = dram.tile([EDRAM_ROWS, D], BF16)
            # compact per-round staging (round r>0 gathers its sources from
            # round r-1's staging, off the E_dram scatter critical path)
            E_stage = [None]
            for r in range(1, len(round_tiles)):
                E_stage.append(dram.tile([round_tiles[r] * 128, D], BF16,
                                         name=f"E_stage{r}"))
            ar_in = dram.tile([1, 1], F32)
            ar_out = dram.tile([N_CORES, 1], F32, addr_space="Shared")

            persist = top.enter_context(tc.tile_pool(name="persist", bufs=1))
            # AT: full transposed matrix, [feat%128, feat//128, unified col] bf16
            AT = persist.tile([128, KCH, CEXT], BF16)
            # LHS: own rows transposed, [feat%128, feat//128, own row] bf16
            LHS = persist.tile([128, KCH, OWN], BF16)
            ident = persist.tile([128, 128], BF16)
            make_identity(nc, ident[:])
            ones = persist.tile([128, 1], F32)
            nc.vector.memset(ones[:], 1.0)
            rowparts = persist.tile([128, NT_OWN, PANELS], F32)
            dlog = persist.tile([128, NT_OWN], F32)
            dexp = persist.tile([128, NT_OWN], F32)

            # index/mask inputs -> SBUF
            idxp = top.enter_context(tc.tile_pool(name="idx", bufs=1))
            zmask_sb = idxp.tile([128, C // 128], F32)
            nc.sync.dma_start(out=zmask_sb[:], in_=zmask[:, :])
            gp_sb = idxp.tile([128, NT_OWN], I32)
            gs_sb = idxp.tile([128, NT_OWN], I32)
            m1_sb = idxp.tile([128, NT_OWN], F32)
            m2_sb = idxp.tile([128, NT_OWN], F32)
            m0_sb = idxp.tile([128, round_tiles[0]], F32)
            nc.scalar.dma_start(out=gp_sb[:], in_=gp[:, :])
            nc.scalar.dma_start(out=gs_sb[:], in_=gs[:, :])
            nc.scalar.dma_start(out=m1_sb[:], in_=m1[:, :])
            nc.scalar.dma_start(out=m2_sb[:], in_=m2[:, :])
            nc.scalar.dma_start(out=m0_sb[:], in_=m0[:, :])
            ridx_sb = []
            for r, (g, f, s) in enumerate(ridx):
                nt = round_tiles[r]
                gsb = idxp.tile([128, nt], I32, name=f"g{r}sb")
                fsb = idxp.tile([128, nt], I32, name=f"f{r}sb")
                ssb = idxp.tile([128, nt], I32, name=f"s{r}sb")
                eng = nc.sync if r == 0 else nc.scalar
                eng.dma_start(out=gsb[:], in_=g[:, :])
                eng.dma_start(out=fsb[:], in_=f[:, :])
                eng.dma_start(out=ssb[:], in_=s[:, :])
                ridx_sb.append((gsb, fsb, ssb))

            def body():
                with ExitStack() as ph:
                    psv = ph.enter_context(
                        tc.tile_pool(name="psV", bufs=2, space="PSUM"))
                    sbe = ph.enter_context(tc.tile_pool(name="ema", bufs=2))
                    sbv = ph.enter_context(tc.tile_pool(name="phaseV", bufs=2))
                    pmm = ph.enter_context(
                        tc.tile_pool(name="psMM", bufs=3, space="PSUM"))

                    # ------------- Phase Z: zero E_dram trash region -----------
                    # (rows 0..SLOT are fully written by r0's direct stores,
                    # with pad rows zeroed via the m0 mask)
                    zeros_t = sbv.tile([128, D], BF16, tag="ck", name="zeros_t")
                    nc.vector.memset(zeros_t[:], 0.0)
                    for t in range(round_tiles[0], EDRAM_ROWS // 128):
                        nc.scalar.dma_start(
                            out=E_dram[t * 128:(t + 1) * 128, :], in_=zeros_t[:])

                    def ema_gather(r, t):
                        # indirect gathers for one EMA tile
                        gsb, fsb, ssb = ridx_sb[r]
                        if r == 0:
                            G = sbe.tile([128, D], F32, tag="gema",
                                         name=f"g_{r}_{t}")
                            nc.gpsimd.indirect_dma_start(
                                out=G[:, :], out_offset=None,
                                in_=protos[:, :],
                                in_offset=IndirectOffsetOnAxis(
                                    ap=gsb[:, t:t + 1], axis=0),
                            )
                        else:
                            G = sbe.tile([128, D], BF16, tag="mn",
                                         name=f"g_{r}_{t}")
                            gsrc = E_dram[0:SLOT, :] if r == 1 \
                                else E_stage[r - 1][:, :]
                            nc.gpsimd.indirect_dma_start(
                                out=G[:, :], out_offset=None,
                                in_=gsrc,
                                in_offset=IndirectOffsetOnAxis(
                                    ap=gsb[:, t:t + 1], axis=0),
                            )
                        Ft = sbe.tile([128, D], F32, tag="fema",
                                      name=f"f_{r}_{t}")
                        nc.gpsimd.indirect_dma_start(
                            out=Ft[:, :], out_offset=None,
                            in_=feats[:, :],
                            in_offset=IndirectOffsetOnAxis(
                                ap=fsb[:, t:t + 1], axis=0),
                        )
                        return G, Ft

                    def ema_compute(r, t, G, Ft):
                        # normalize + scatter for one EMA tile.  Buffer
                        # economy: Ft is squashed in place into m; G doubles
                        # as the m^2 dump (r0).
                        gsb, fsb, ssb = ridx_sb[r]
                        # m = f*(0.05/0.95) + g (positive scale of ref row;
                        # removed by the normalize)
                        nc.vector.scalar_tensor_tensor(
                            out=Ft[:], in0=Ft[:], scalar=0.05 / 0.95,
                            in1=G[:], op0=OP.mult, op1=OP.add,
                        )
                        if r == 0:
                            Msq = G  # G is dead; reuse as the square dump
                        else:
                            Msq = sbe.tile([128, D], F32, tag="gema",
                                           name=f"msq_{r}_{t}")
                        ssq = sbe.tile([128, 1], F32, tag="ssq")
                        nc.vector.scalar_tensor_tensor(
                            out=Msq[:], in0=Ft[:], scalar=1.0, in1=Ft[:],
                            op0=OP.mult, op1=OP.mult, accum_out=ssq[:],
                        )
                        rsq = sbe.tile([128, 1], F32, tag="rsq")
                        nc.vector.reciprocal(rsq[:], ssq[:])
                        rnorm = sbe.tile([128, 1], F32, tag="rnorm")
                        nc.scalar.activation(rnorm[:], rsq[:], AF.Sqrt)
                        Mn = sbe.tile([128, D], BF16, tag="mn",
                                      name=f"mn_{r}_{t}")
                        if r == 0:
                            # r0 slots are contiguous (first-occurrence
                            # order): zero the pad rows via m0 and store with
                            # a plain direct DMA instead of a scatter
                            nc.vector.tensor_scalar(
                                out=Mn[:], in0=Ft[:], scalar1=rnorm[:],
                                scalar2=m0_sb[:, t:t + 1],
                                op0=OP.mult, op1=OP.mult,
                            )
                            nc.scalar.dma_start(
                                out=E_dram[t * 128:(t + 1) * 128, :],
                                in_=Mn[:])
                        else:
                            nc.vector.tensor_scalar_mul(Mn[:], Ft[:], rnorm[:])
                            # compact staging feeds the next round directly;
                            # the E_dram scatter is off that critical path
                            nc.scalar.dma_start(
                                out=E_stage[r][t * 128:(t + 1) * 128, :],
                                in_=Mn[:])
                            nc.gpsimd.indirect_dma_start(
                                out=E_dram[0:TRASH + 1, :],
                                out_offset=IndirectOffsetOnAxis(
                                    ap=ssb[:, t:t + 1], axis=0),
                                in_=Mn[:, :], in_offset=None,
                            )

                    def ema_round(r):
                        pend = []
                        for t in range(round_tiles[r]):
                            pend.append((t,) + ema_gather(r, t))
                            if len(pend) > 1:
                                tt, G, Ft = pend.pop(0)
                                ema_compute(r, tt, G, Ft)
                        for tt, G, Ft in pend:
                            ema_compute(r, tt, G, Ft)

                    def transpose_tile(src_bf16, dst, dst_base, eng=nc.scalar):
                        # 8 PE transposes -> 1 psum [128,8,128] -> 1 strided copy
                        pt = psv.tile([128, KCH, 128], BF16, tag="pt")
                        for k in range(KCH):
                            nc.tensor.transpose(
                                pt[:, k, :],
                                src_bf16[:, k * 128:(k + 1) * 128],
                                ident[:],
                            )
                        dstv = dst[:, :, dst_base:dst_base + 128]
                        if eng is nc.vector:
                            nc.vector.tensor_copy(dstv, pt[:, :, :])
                        else:
                            nc.scalar.copy(dstv, pt[:, :, :])

                    def stage_tile(p, i):
                        # load + convert + transpose for tile i of panel p
                        if True:
                            t = p * 8 + i
                            if t < C // 128:
                                Lk = sbv.tile([128, D], F32, tag="lk")
                                nc.sync.dma_start(
                                    out=Lk[:],
                                    in_=protos[t * 128:(t + 1) * 128, :])
                                Ck = sbv.tile([128, D], BF16, tag="ck")
                                nc.scalar.mul(Ck[:], Lk[:],
                                              zmask_sb[:, t:t + 1])
                                transpose_tile(Ck, AT, t * 128,
                                               eng=nc.vector)
                            else:
                                ts = t - C // 128
                                St = sbv.tile([128, D], BF16, tag="ck",
                                              name=f"st{ts}")
                                nc.gpsimd.dma_start(
                                    out=St[:],
                                    in_=E_dram[ts * 128:(ts + 1) * 128, :])
                                transpose_tile(St, AT, t * 128,
                                               eng=nc.vector)

                    def stage(p):
                        for i in range(8):
                            stage_tile(p, i)

                    def phase_L():
                        # merged own rows -> LHS + diag (tiles alias phaseV /
                        # ema pool tags; phase L strictly follows EMA)
                        for t in range(NT_OWN):
                            A = sbv.tile([128, D], F32, tag="lk",
                                         name=f"argh{t}")
                            nc.gpsimd.indirect_dma_start(
                                out=A[:, :], out_offset=None,
                                in_=protos[:, :],
                                in_offset=IndirectOffsetOnAxis(
                                    ap=gp_sb[:, t:t + 1], axis=0),
                            )
                            Bt = sbv.tile([128, D], BF16, tag="ck",
                                          name=f"bgh{t}")
                            nc.gpsimd.indirect_dma_start(
                                out=Bt[:, :], out_offset=None,
                                in_=E_dram[:, :],
                                in_offset=IndirectOffsetOnAxis(
                                    ap=gs_sb[:, t:t + 1], axis=0),
                            )
                            # bf16(A*m1) matches the AT path's bf16(zmask*row)
                            T1 = sbe.tile([128, D], BF16, tag="mn",
                                          name=f"t1_{t}")
                            nc.vector.tensor_scalar_mul(
                                T1[:], A[:], m1_sb[:, t:t + 1])
                            Mg = sbe.tile([128, D], BF16, tag="mn",
                                          name=f"mg{t}")
                            nc.vector.scalar_tensor_tensor(
                                out=Mg[:], in0=Bt[:],
                                scalar=m2_sb[:, t:t + 1],
                                in1=T1[:], op0=OP.mult, op1=OP.add,
                            )
                            transpose_tile(Mg, LHS, t * 128, eng=nc.vector)
                            # diag: |row|^2 of the same bf16 values (A is dead;
                            # reuse as the square dump, only the accum matters)
                            nc.vector.scalar_tensor_tensor(
                                out=A[:], in0=Mg[:], scalar=1.0, in1=Mg[:],
                                op0=OP.mult, op1=OP.mult,
                                accum_out=dlog[:, t:t + 1],
                            )

                    def mm_panel(p, stage_p=None):
                        # matmuls + exp row-sums for column panel p; the next
                        # panel's stage tiles interleave between matmul pairs
                        # so their transposes never starve PE
                        for j, b0 in enumerate(range(0, NT_OWN, 2)):
                            pair = [b0] if b0 + 1 >= NT_OWN else [b0, b0 + 1]
                            pss = []
                            for bi in pair:
                                ps = pmm.tile([128, 1024], F32, tag="ps",
                                              name=f"ps_{p}_{bi}")
                                pss.append(ps)
                            for k in range(KCH):
                                # 512-wide matmuls (ISA cap), 2x2 interleaved
                                # accumulation chains to hide psum latency
                                for h in range(2):
                                    for ps, bi in zip(pss, pair):
                                        nc.tensor.matmul(
                                            ps[:, h * 512:(h + 1) * 512],
                                            LHS[:, k, bi * 128:(bi + 1) * 128],
                                            AT[:, k, p * 1024 + h * 512:
                                               p * 1024 + (h + 1) * 512],
                                            start=(k == 0),
                                            stop=(k == KCH - 1),
                                        )
                            for ps, bi in zip(pss, pair):
                                # exp in place on the psum tile; only the
                                # accumulated row-sum is consumed
                                nc.scalar.activation(
                                    ps[:], ps[:], AF.Exp, scale=INV_TEMP,
                                    accum_out=rowparts[:, bi, p:p + 1],
                                )
                            if stage_p is not None:
                                for i in (2 * j, 2 * j + 1):
                                    if i < 8:
                                        stage_tile(stage_p, i)

                    # Emission: EMA rounds first (gathers pipelined one tile
                    # ahead of computes so scatters never head-block the
                    # gpsimd queue); stage 0 rides the DMA queue behind r0;
                    # phase L gates the first matmul; stages 1+ interleave
                    # with the matmul panels so their DMA hides under PE
                    # streaming.
                    if parts != "full":
                        # timing-bisection builds: initialize tiles the
                        # skipped phases would have written (never taken by
                        # the graded kernel() path, which uses parts="full")
                        nc.vector.memset(dlog[:], 1.0)
                        nc.vector.memset(rowparts[:, :, :], 1.0)
                        for _k in range(KCH):
                            nc.vector.memset(LHS[:, _k, :], 0.0)
                            nc.vector.memset(AT[:, _k, :], 0.0)

                    # r0 tiles interleave with stage-0 tiles (r0 has no
                    # gpsimd scatters, so nothing head-blocks); gathers run
                    # one tile ahead of computes
                    if parts in ("full", "prefix"):
                        pend = []
                        for i in range(8):
                            if i < round_tiles[0]:
                                pend.append((i,) + ema_gather(0, i))
                            if len(pend) > 1:
                                tt, G, Ft = pend.pop(0)
                                ema_compute(0, tt, G, Ft)
                            stage_tile(0, i)
                        for tt, G, Ft in pend:
                            ema_compute(0, tt, G, Ft)
                        for r in range(1, len(round_tiles)):
                            ema_round(r)
                        phase_L()
                    if parts == "full":
                        for p in range(PANELS):
                            mm_panel(p, stage_p=p + 1 if p + 1 < PANELS
                                     else None)
                    elif parts == "mm":
                        for p in range(PANELS):
                            mm_panel(p, stage_p=None)
                    else:
                        for p in range(1, PANELS):
                            stage(p)
                    nc.scalar.activation(dexp[:], dlog[:], AF.Exp,
                                         scale=INV_TEMP)

                # ---------------- Phase F: local reduce ------------------------
                with ExitStack() as ph:
                    fin = ph.enter_context(tc.tile_pool(name="fin", bufs=1))
                    pfin = ph.enter_context(
                        tc.tile_pool(name="psFin", bufs=1, space="PSUM"))
                    rs = fin.tile([128, NT_OWN], F32)
                    nc.vector.tensor_reduce(
                        rs[:], rowparts[:, :, :], axis=AX.X, op=OP.add)
                    rmask = fin.tile([128, NT_OWN], F32)
                    nc.vector.tensor_add(rmask[:], m1_sb[:], m2_sb[:])
                    sn = fin.tile([128, NT_OWN], F32)
                    nc.vector.tensor_sub(sn[:], rs[:], dexp[:])
                    sn2 = fin.tile([128, NT_OWN], F32)
                    nc.vector.tensor_scalar_add(sn2[:], sn[:], -float(SLOT))
                    lg = fin.tile([128, NT_OWN], F32)
                    nc.scalar.activation(lg[:], sn2[:], AF.Ln)
                    lgm = fin.tile([128, NT_OWN], F32)
                    nc.vector.tensor_mul(lgm[:], lg[:], rmask[:])
                    l1 = fin.tile([128, 1], F32)
                    nc.vector.reduce_sum(l1[:], lgm[:], axis=AX.X)
                    pt1 = pfin.tile([1, 1], F32)
                    nc.tensor.matmul(pt1[:], ones[:], l1[:], start=True, stop=True)
                    tot = fin.tile([1, 1], F32)
                    nc.vector.tensor_copy(tot[:], pt1[:])
                    nc.sync.dma_start(out=ar_in[:, :], in_=tot[:])

            if loop_k is not None:
                with tc.For_i(0, loop_k, 1):
                    body()
            else:
                body()

            # ------------- AllGather partials + local sum + final scale ----
            nc.gpsimd.collective_compute(
                "AllGather", mybir.AluOpType.bypass,
                replica_groups=[list(range(N_CORES))],
                ins=[ar_in.opt()], outs=[ar_out.opt()],
            )
            with ExitStack() as ph:
                fin2 = ph.enter_context(tc.tile_pool(name="fin2", bufs=1))
                ld = fin2.tile([1, N_CORES], F32)
                nc.sync.dma_start(
                    out=ld[:], in_=ar_out[:, :].rearrange("a b -> b a"))
                tot8 = fin2.tile([1, 1], F32)
                nc.vector.reduce_sum(tot8[:], ld[:], axis=mybir.AxisListType.X)
                fv = fin2.tile([1, 1], F32)
                # loss = total/C - log(C-1)
                nc.vector.tensor_scalar(
                    out=fv[:], in0=tot8[:], scalar1=1.0 / C,
                    scalar2=-math.log(C - 1), op0=mybir.AluOpType.mult,
                    op1=mybir.AluOpType.add,
                )
                nc.sync.dma_start(out=out[:, :], in_=fv[:])

    nc.finalize()
    return nc


def host_prep(labels):
    """Host-side index/mask prep (global EMA rounds + per-core row shards)."""
    labels = np.asarray(labels).astype(np.int64)
    Bn = labels.shape[0]

    slot_of = {}          # label -> slot (first-occurrence order)
    occ = {}              # label -> #occurrences seen so far
    rounds = []           # rounds[r] = list of (slot, label, sample_idx)
    for i, l in enumerate(labels.tolist()):
        r = occ.get(l, 0)
        occ[l] = r + 1
        if l not in slot_of:
            slot_of[l] = len(slot_of)
        while len(rounds) <= r:
            rounds.append([])
        rounds[r].append((slot_of[l], l, i))
    u = len(slot_of)
    n_rounds = len(rounds)

    round_tiles = [max(1, (len(rounds[r]) + 127) // 128) for r in range(n_rounds)]

    def col_major(vec, nt):
        return np.ascontiguousarray(
            np.asarray(vec, dtype=np.int32).reshape(nt, 128).T)

    glob = {}
    stale = np.zeros(C, dtype=bool)
    for l in slot_of:
        stale[l] = True
    zm = np.where(stale, 0.0, 1.0).astype(np.float32)
    glob["zmask"] = np.ascontiguousarray(zm.reshape(C // 128, 128).T)

    for r in range(n_rounds):
        L = round_tiles[r] * 128
        g = np.zeros(L, dtype=np.int32)
        f = np.zeros(L, dtype=np.int32)
        s = np.full(L, TRASH, dtype=np.int32)
        if r >= 2:
            # position of each label within the previous round's staging
            pos_prev = {l: j for j, (_, l, _) in enumerate(rounds[r - 1])}
        for j, (slot, l, i) in enumerate(rounds[r]):
            if r == 0:
                g[j] = l          # prototypes row
            elif r == 1:
                g[j] = slot       # E_dram row (round-0 result)
            else:
                g[j] = pos_prev[l]  # E_stage[r-1] row
            f[j] = i
            s[j] = slot
        glob[f"g{r}"] = col_major(g, round_tiles[r])
        glob[f"f{r}"] = col_major(f, round_tiles[r])
        glob[f"s{r}"] = col_major(s, round_tiles[r])

    L0 = round_tiles[0] * 128
    m0v = (np.arange(L0) < len(rounds[0])).astype(np.float32)
    glob["m0"] = np.ascontiguousarray(
        m0v.reshape(round_tiles[0], 128).T)

    per_core = []
    for c in range(N_CORES):
        gp = np.zeros(OWN, dtype=np.int32)
        gs = np.full(OWN, TRASH, dtype=np.int32)
        m1v = np.zeros(OWN, dtype=np.float32)
        m2v = np.zeros(OWN, dtype=np.float32)
        for j in range(OWN):
            row = c * OWN + j
            if row < C:
                gp[j] = row
                if not stale[row]:
                    m1v[j] = 1.0
            else:
                slot = row - C
                gs[j] = slot
                if slot < u:
                    m2v[j] = 1.0
        d = dict(glob)
        d["gp"] = col_major(gp, NT_OWN)
        d["gs"] = col_major(gs, NT_OWN)
        d["m1"] = np.ascontiguousarray(
            m1v.reshape(NT_OWN, 128).T.astype(np.float32))
        d["m2"] = np.ascontiguousarray(
            m2v.reshape(NT_OWN, 128).T.astype(np.float32))
        per_core.append(d)
    return round_tiles, per_core


_NC_CACHE = {}


def kernel(features, labels, prototypes):
    _shim_axon_hooks()
    from concourse.bass_utils import run_bass_kernel_spmd

    features = np.ascontiguousarray(np.asarray(features), dtype=np.float32)
    prototypes = np.ascontiguousarray(np.asarray(prototypes), dtype=np.float32)
    round_tiles, per_core = host_prep(labels)

    key = tuple(round_tiles)
    if key not in _NC_CACHE:
        _NC_CACHE[key] = build_nc(round_tiles)
    nc = _NC_CACHE[key]

    in_maps = []
    for c in range(N_CORES):
        m = {"features": features, "prototypes": prototypes}
        m.update(per_core[c])
        in_maps.append(m)

    res = run_bass_kernel_spmd(nc, in_maps, core_ids=list(range(N_CORES)), trace=False)
    val = np.float32(res.results[0]["out"][0, 0])
    return np.asarray(val, dtype=np.float32).reshape(())
